# revision 1
# baseline (speedup 1.0000x reference)
"""Trainium2 Bass kernel for nn_DecoderBlock (B=4, S=1024, E=1024, H=16, D=4096).

Sharding: sequence-data-parallel over 8 NeuronCores, zero collectives.
Core c handles (batch b = c//2, half h = c%2): it computes K/V over the
batch's full 1024-token sequence, Q + attention + FFN for its 512 query
tokens, and returns out[b, h*512:(h+1)*512, :]. Causality is handled by a
per-core additive mask tensor so all 8 cores run one SPMD program.

All matmuls run in float32r (TF32-like, full PE rate) with fp32 PSUM
accumulation. Activations live transposed [features, tokens] on-chip so
every matmul consumes natural-layout weights as the stationary operand;
LayerNorm runs in natural layout with PE transposes in between (gamma/beta
and FFN biases fold into per-partition fused ops in transposed space).
Softmax skips max-subtraction (scores are O(sigma=1); exp cannot overflow)
and gets its denominator from a ones-column appended to the PV weights,
broadcast back via a tiny selection-matrix matmul.
"""

import sys

if "/opt/trn_rl_repo" not in sys.path:
    sys.path.insert(0, "/opt/trn_rl_repo")

import json

import numpy as np

import concourse.bass as bass
import concourse.mybir as mybir
from concourse.tile import TileContext

P = 128
B, S, E = 4, 1024, 1024
H, KD = 16, 64
D = 4096
TQ = 512  # query tokens per core
ES = E // P  # 8
DS = D // P  # 32
KO = S // P  # 8
NQ = TQ // P  # 4
PAIRS = H // 2  # 8
EPS = 1e-5
NEG = -1.0e30

F32 = mybir.dt.float32
F32R = mybir.dt.float32r
AF = mybir.ActivationFunctionType
OP = mybir.AluOpType


# ---------------------------------------------------------------------------
# BIR post-pass: this container's walrus accepts only one sync-wait command
# per instruction, but Tile's exit Drain aggregates one wait per engine/DMA
# queue. Split multi-wait instructions into preceding single-wait NoOps.
# ---------------------------------------------------------------------------
def _fix_bir_json(j):
    counter = 0
    changed = False
    for fn in j.get("functions", []):
        for blk in fn.get("blocks", []):
            out = []
            for inst in blk.get("instructions", []):
                si = inst.get("sync_info") or {}
                waits = si.get("on_wait") or []
                if len(waits) > 1:
                    changed = True
                    for w in waits[:-1]:
                        counter += 1
                        out.append(
                            {
                                "debug": inst.get("debug", 0),
                                "engine": inst["engine"],
                                "ins": [],
                                "name": f"WFIX-{counter}",
                                "opcode": "NoOp",
                                "outs": [],
                                "sync_info": {"on_update": [], "on_wait": [w]},
                            }
                        )
                    si["on_wait"] = waits[-1:]
                    inst["sync_info"] = si
                out.append(inst)
            blk["instructions"] = out
    return changed


class PatchedBass(bass.Bass):
    def to_json_bytes(self):
        raw = super().to_json_bytes()
        j = json.loads(raw)
        if _fix_bir_json(j):
            return json.dumps(j).encode()
        return raw


# ---------------------------------------------------------------------------
# Program builder (one SPMD program shared by all 8 cores)
# ---------------------------------------------------------------------------
def build_program(debug=False):
    nc = PatchedBass()

    # --- DRAM tensors (per-core values supplied via in_maps) ---
    x_kv = nc.dram_tensor("x_kv", [S, E], F32, kind="ExternalInput")
    x_q = nc.dram_tensor("x_q", [TQ, E], F32, kind="ExternalInput")
    maskt = nc.dram_tensor("maskt", [P, KO * TQ], F32R, kind="ExternalInput")
    trilc = nc.dram_tensor("trilc", [P, P], F32R, kind="ExternalInput")
    wq = nc.dram_tensor("wq", [ES, P, ES * P], F32R, kind="ExternalInput")
    wk = nc.dram_tensor("wk", [ES, P, ES * P], F32R, kind="ExternalInput")
    wv = nc.dram_tensor("wv", [4, P, ES * 256], F32R, kind="ExternalInput")
    projw = nc.dram_tensor("projw", [ES, P, ES * P], F32R, kind="ExternalInput")
    finw = nc.dram_tensor("finw", [DS, P, ES * P], F32R, kind="ExternalInput")
    hidw = nc.dram_tensor("hidw", [2, DS, 4, P, ES * P], F32R, kind="ExternalInput")
    foutw = nc.dram_tensor("foutw", [ES, 4, P, ES * P], F32R, kind="ExternalInput")
    ident = nc.dram_tensor("ident", [P, P], F32R, kind="ExternalInput")
    onesc = nc.dram_tensor("onesc", [P, 64], F32R, kind="ExternalInput")
    g1c = nc.dram_tensor("g1c", [P, ES], F32, kind="ExternalInput")
    b1c = nc.dram_tensor("b1c", [P, ES], F32, kind="ExternalInput")
    g2c = nc.dram_tensor("g2c", [P, ES], F32, kind="ExternalInput")
    b2c = nc.dram_tensor("b2c", [P, ES], F32, kind="ExternalInput")
    projb = nc.dram_tensor("projb", [P, ES], F32, kind="ExternalInput")
    finb = nc.dram_tensor("finb", [P, DS], F32, kind="ExternalInput")
    hidb = nc.dram_tensor("hidb", [P, 2 * DS], F32, kind="ExternalInput")
    foutb = nc.dram_tensor("foutb", [P, ES], F32, kind="ExternalInput")
    out = nc.dram_tensor("out", [TQ, E], F32, kind="ExternalOutput")
    dbg = {}
    if debug:
        for nm, shp in [("d_hnT", [E, S]), ("d_ktt", [E, S]), ("d_qtt", [E, TQ]),
                        ("d_vp", [S, H * 65]), ("d_ott", [E, TQ]), ("d_x1", [TQ, E]),
                        ("d_yt", [E, TQ]), ("d_ft1", [D, TQ]), ("d_ft2", [D, TQ]),
                        ("d_ft3", [D, TQ]), ("d_outt", [E, TQ]),
                        ("d_finb", [P, DS]), ("d_hidb", [P, 2 * DS]),
                        ("d_projb", [P, ES]), ("d_foutb", [P, ES]),
                        ("d_g2", [P, ES])]:
            dbg[nm] = nc.dram_tensor(nm, shp, F32, kind="ExternalOutput")

    with TileContext(nc) as tc:
        # ---- global pools (whole kernel) ----
        pools = []

        def open_pool(**kw):
            cm = tc.tile_pool(**kw)
            pool = cm.__enter__()
            return cm, pool

        cp_cm, cp = open_pool(name="const", bufs=1)
        small_cm, small = open_pool(name="small", bufs=2)
        scr_cm, scrp = open_pool(name="scr", bufs=1)
        xt_cm, xtp = open_pool(name="xt", bufs=2)
        xn_cm, xnp = open_pool(name="xn", bufs=2)
        w_cm, wp = open_pool(name="w", bufs=5)
        dram_cm, dramp = open_pool(name="dram", bufs=1, space="DRAM")
        ps_cm, ps = open_pool(name="ps", bufs=3, space="PSUM")
        tp_cm, tp = open_pool(name="tp", bufs=2, space="PSUM")
        pools += [cp_cm, small_cm, scr_cm, xt_cm, xn_cm, w_cm, dram_cm, ps_cm, tp_cm]

        # constants
        t_ident = cp.tile([P, P], F32R, tag="ident")
        nc.sync.dma_start(t_ident[:], ident[:])
        t_ones = cp.tile([P, 64], F32R, tag="ones")
        nc.sync.dma_start(t_ones[:], onesc[:])
        t_tril = cp.tile([P, P], F32R, tag="tril")
        nc.sync.dma_start(t_tril[:], trilc[:])
        t_g1 = cp.tile([P, ES], F32, tag="g1")
        nc.sync.dma_start(t_g1[:], g1c[:])
        t_b1 = cp.tile([P, ES], F32, tag="b1")
        nc.sync.dma_start(t_b1[:], b1c[:])
        t_eps = cp.tile([P, 1], F32, tag="eps")
        nc.vector.memset(t_eps[:], EPS)

        # LayerNorm stats: returns (mean, rstd) [P,1] tiles
        def ln_stats(xt):
            scr = scrp.tile([P, E], F32, tag="scr")
            s1 = small.tile([P, 1], F32, tag="s1")
            s2 = small.tile([P, 1], F32, tag="s2")
            nc.scalar.activation(scr[:], xt[:], AF.Copy, accum_out=s1[:])
            nc.scalar.activation(scr[:], xt[:], AF.Square, accum_out=s2[:])
            m = small.tile([P, 1], F32, tag="m")
            nc.vector.tensor_scalar_mul(m[:], s1[:], 1.0 / E)
            var = small.tile([P, 1], F32, tag="var")
            nc.vector.tensor_scalar_mul(var[:], s2[:], 1.0 / E)
            m2 = small.tile([P, 1], F32, tag="m2")
            nc.vector.tensor_tensor(m2[:], m[:], m[:], OP.mult)
            nc.vector.tensor_tensor(var[:], var[:], m2[:], OP.subtract)
            sd = small.tile([P, 1], F32, tag="sd")
            nc.scalar.activation(sd[:], var[:], AF.Sqrt, bias=t_eps[:])
            rstd = small.tile([P, 1], F32, tag="rstd")
            nc.vector.reciprocal(rstd[:], sd[:])
            return m, rstd

        # LN a natural [P,E] tile/AP then transpose into dstT[:, es, col0:col0+P]
        # fusing per-feature gamma/beta (per-partition in transposed space).
        def ln_transpose(xt, dstT, col0, tg, tb):
            m, rstd = ln_stats(xt)
            xn = xnp.tile([P, E], F32R, tag="xn")
            nc.vector.tensor_scalar(xn[:], xt[:], m[:], rstd[:], OP.subtract, OP.mult)
            for es in range(ES):
                ptp = tp.tile([P, P], F32R, tag="tp")
                nc.tensor.transpose(ptp[:], xn[:, es * P : (es + 1) * P], t_ident[:])
                nc.vector.tensor_scalar(
                    dstT[:, es, col0 : col0 + P],
                    ptp[:],
                    tg[:, es : es + 1],
                    tb[:, es : es + 1],
                    OP.mult,
                    OP.add,
                )

        x1s = dramp.tile([TQ, E], F32, tag="x1s")

        # ================= Phase A: attention =================
        ot_sb_cm, otsb = open_pool(name="otsb", bufs=1)  # OTt lives all of A
        OTt = otsb.tile([P, PAIRS, TQ], F32R, tag="OTt")

        stpv_cm, stpv = open_pool(name="stpv", bufs=1)  # KT/QT/V live A1+A2
        KTt = stpv.tile([P, PAIRS, S], F32R, tag="KTt")
        QTt = stpv.tile([P, PAIRS, TQ], F32R, tag="QTt")
        Vp = stpv.tile([P, KO, H * 65], F32R, tag="Vp")

        # ---- A1: LN1 + QKV projections ----
        hn_cm, hp = open_pool(name="hn", bufs=1)
        wv_cm, wvp = open_pool(name="wv", bufs=1)

        hnT = hp.tile([P, ES, S], F32R, tag="hnT")
        for tko in range(KO):
            xt = xtp.tile([P, E], F32, tag="xt")
            nc.sync.dma_start(xt[:], x_kv[tko * P : (tko + 1) * P, :])
            ln_transpose(xt, hnT, tko * P, t_g1, t_b1)

        t_g2 = cp.tile([P, ES], F32, tag="g2")
        nc.sync.dma_start(t_g2[:], g2c[:])
        t_b2 = cp.tile([P, ES], F32, tag="b2")
        nc.sync.dma_start(t_b2[:], b2c[:])
        t_projb = cp.tile([P, ES], F32, tag="projb")
        nc.sync.dma_start(t_projb[:], projb[:])
        t_finb = cp.tile([P, DS], F32, tag="finb")
        nc.sync.dma_start(t_finb[:], finb[:])
        t_hidb = cp.tile([P, 2 * DS], F32, tag="hidb")
        nc.sync.dma_start(t_hidb[:], hidb[:])
        t_foutb = cp.tile([P, ES], F32, tag="foutb")
        nc.sync.dma_start(t_foutb[:], foutb[:])

        # KT = wk.T @ hnT  -> [h*kd, tk]  (nh outer: start after half the LN)
        for nh in range(2):
            for mi in range(ES):
                wc = wp.tile([P, ES, P], F32R, tag="w")
                nc.sync.dma_start(wc[:], wk[mi].rearrange("p (e j) -> p e j", e=ES))
                psm = ps.tile([P, 512], F32, tag="ps")
                for es in range(ES):
                    nc.tensor.matmul(
                        psm[:],
                        wc[:, es, :],
                        hnT[:, es, nh * 512 : (nh + 1) * 512],
                        start=(es == 0),
                        stop=(es == ES - 1),
                    )
                nc.scalar.activation(
                    KTt[:, mi, nh * 512 : (nh + 1) * 512], psm[:], AF.Identity
                )

        # V (natural layout + ones column): lhsT = hnT chunk, rhs = wv quarter
        for jq in range(4):
            wvq = wvp.tile([P, ES, 256], F32R, tag="wv")
            nc.sync.dma_start(wvq[:], wv[jq].rearrange("p (e j) -> p e j", e=ES))
            for tko in range(KO):
                psm = ps.tile([P, 512], F32, tag="ps")
                for es in range(ES):
                    nc.tensor.matmul(
                        psm[:, 0:256],
                        hnT[:, es, tko * P : (tko + 1) * P],
                        wvq[:, es, :],
                        start=(es == 0),
                        stop=(es == ES - 1),
                    )
                vview = Vp[:, tko, :].rearrange("p (h c) -> p h c", c=65)
                nc.vector.tensor_copy(
                    vview[:, jq * 4 : (jq + 1) * 4, 0:64],
                    psm[:, 0:256].rearrange("p (h c) -> p h c", c=64),
                )
        for tko in range(KO):
            vview = Vp[:, tko, :].rearrange("p (h c) -> p h c", c=65)
            nc.sync.dma_start(
                vview[:, :, 64:65],
                onesc[:, 0:16].rearrange("p (h c) -> p h c", c=1),
            )

        if debug:
            for es in range(ES):
                nc.sync.dma_start(dbg["d_hnT"][es * P:(es + 1) * P, :], hnT[:, es, :].bitcast(F32))
            for pr in range(PAIRS):
                nc.sync.dma_start(dbg["d_ktt"][pr * P:(pr + 1) * P, :], KTt[:, pr, :].bitcast(F32))
            for tko in range(KO):
                nc.sync.dma_start(dbg["d_vp"][tko * P:(tko + 1) * P, :], Vp[:, tko, :].bitcast(F32))

        wv_cm.__exit__(None, None, None)
        hn_cm.__exit__(None, None, None)

        # ---- A1b: Q projection (own scope, after hnT freed) ----
        hq_cm, hqp = open_pool(name="hq", bufs=1)
        # QT = wq.T @ LN1(x_q)^T, in two 256-token halves
        hnQ2 = hqp.tile([P, ES, 256], F32R, tag="hnQ2")
        for qh in range(2):
            for qj in range(2):
                qi = qh * 2 + qj
                xt = xtp.tile([P, E], F32, tag="xt")
                nc.sync.dma_start(xt[:], x_q[qi * P : (qi + 1) * P, :])
                ln_transpose(xt, hnQ2, qj * P, t_g1, t_b1)
            for mi in range(ES):
                wc = wp.tile([P, ES, P], F32R, tag="w")
                nc.sync.dma_start(wc[:], wq[mi].rearrange("p (e j) -> p e j", e=ES))
                psm = ps.tile([P, 512], F32, tag="ps")
                for es in range(ES):
                    nc.tensor.matmul(
                        psm[:, 0:256],
                        wc[:, es, :],
                        hnQ2[:, es, :],
                        start=(es == 0),
                        stop=(es == ES - 1),
                    )
                nc.scalar.activation(
                    QTt[:, mi, qh * 256 : (qh + 1) * 256], psm[:, 0:256], AF.Identity
                )

        if debug:
            for pr in range(PAIRS):
                nc.sync.dma_start(dbg["d_qtt"][pr * P:(pr + 1) * P, :], QTt[:, pr, :].bitcast(F32))
        hq_cm.__exit__(None, None, None)


        # ---- A2: scores, softmax, PV per head pair ----
        a2_cm, a2 = open_pool(name="a2", bufs=1)
        pt_cm, ptp_pool = open_pool(name="pt", bufs=4)
        lp_cm, lpp = open_pool(name="lp", bufs=2)
        ot_cm, otp = open_pool(name="otps", bufs=3, space="PSUM")

        t_mask = a2.tile([P, KO, TQ], F32R, tag="mask")
        nc.sync.dma_start(t_mask[:], maskt[:].rearrange("p (k t) -> p k t", k=KO))

        for pr in range(PAIRS):
            ots = [
                otp.tile([P, TQ], F32, tag="ot", name=f"ot_{pr}_0"),
                otp.tile([P, TQ], F32, tag="ot", name=f"ot_{pr}_1"),
            ]
            for ko in range(KO):
                for o in range(2):
                    lo, hi = 64 * o, 64 * o + 64
                    st = ps.tile([P, 512], F32, tag="ps")
                    nc.tensor.matmul(
                        st[:],
                        KTt[lo:hi, pr, ko * P : (ko + 1) * P],
                        QTt[lo:hi, pr, :],
                        start=True,
                        stop=False,
                    )
                    nc.tensor.matmul(
                        st[:],
                        t_tril[:],
                        t_mask[:, ko, :],
                        start=False,
                        stop=True,
                    )
                    pt_t = ptp_pool.tile([P, TQ], F32R, tag="pt")
                    nc.scalar.activation(pt_t[:], st[:], AF.Exp, scale=KD**-0.5)
                    h = 2 * pr + o
                    vl = Vp[:, ko, :].rearrange("p (h c) -> p h c", c=65)[:, h, :]
                    nc.tensor.matmul(
                        ots[o][0:65, :],
                        vl,
                        pt_t[:],
                        start=(ko == 0),
                        stop=(ko == KO - 1),
                    )
            # softmax denominators -> ones-row broadcast -> normalized copyback
            for o in range(2):
                lrec = lpp.tile([1, TQ], F32R, tag="lrec", name=f"lrec{pr}_{o}")
                with nc.allow_low_precision(
                    reason="float32r is fp32-width; rounding for matmul input"
                ):
                    nc.vector.reciprocal(lrec[:], ots[o][64:65, :])
                lb = tp.tile([P, 512], F32, tag="tp", name=f"lb{pr}_{o}")
                nc.tensor.matmul(
                    lb[0:64, :], t_ones[0:1, :], lrec[:], start=True, stop=True
                )
                lbs = lpp.tile([64, TQ], F32, tag="lbs", name=f"lbs{pr}_{o}")
                nc.vector.tensor_copy(lbs[:], lb[0:64, :])
                nc.vector.tensor_tensor(
                    OTt[64 * o : 64 * o + 64, pr, :],
                    ots[o][0:64, :],
                    lbs[:],
                    OP.mult,
                )

        if debug:
            for pr in range(PAIRS):
                nc.sync.dma_start(dbg["d_ott"][pr * P:(pr + 1) * P, :], OTt[:, pr, :].bitcast(F32))
        ot_cm.__exit__(None, None, None)
        lp_cm.__exit__(None, None, None)
        pt_cm.__exit__(None, None, None)
        a2_cm.__exit__(None, None, None)
        stpv_cm.__exit__(None, None, None)

        # ---- A3: output projection + residual (x1 -> DRAM scratch) ----
        a3_cm, a3 = open_pool(name="a3", bufs=1)
        attnT = a3.tile([P, ES, TQ], F32R, tag="attnT")
        for mi in range(ES):
            wc = wp.tile([P, ES, P], F32R, tag="w")
            nc.sync.dma_start(wc[:], projw[mi].rearrange("p (e j) -> p e j", e=ES))
            psm = ps.tile([P, 512], F32, tag="ps")
            for es in range(ES):
                nc.tensor.matmul(
                    psm[:],
                    wc[:, es, :],
                    OTt[:, es, :],
                    start=(es == 0),
                    stop=(es == ES - 1),
                )
            nc.scalar.activation(
                attnT[:, mi, :], psm[:], AF.Identity, bias=t_projb[:, mi : mi + 1]
            )
        # x1 = x_q + attn^T  -> DRAM scratch
        for qi in range(NQ):
            xqt = xtp.tile([P, E], F32, tag="xt")
            nc.sync.dma_start(xqt[:], x_q[qi * P : (qi + 1) * P, :])
            x1row = xtp.tile([P, E], F32, tag="x1row")
            for es in range(ES):
                ptp = tp.tile([P, P], F32R, tag="tp")
                nc.tensor.transpose(
                    ptp[:], attnT[:, es, qi * P : (qi + 1) * P], t_ident[:]
                )
                nc.vector.tensor_tensor(
                    x1row[:, es * P : (es + 1) * P],
                    ptp[:],
                    xqt[:, es * P : (es + 1) * P],
                    OP.add,
                )
            nc.sync.dma_start(x1s[qi * P : (qi + 1) * P, :], x1row[:])
            if debug:
                nc.sync.dma_start(dbg["d_x1"][qi * P:(qi + 1) * P, :], x1row[:])
        a3_cm.__exit__(None, None, None)
        ot_sb_cm.__exit__(None, None, None)

        # ================= Phase B: FFN =================
        ffn_cm, fp = open_pool(name="ffn", bufs=1)
        fT1 = fp.tile([P, DS, TQ], F32R, tag="fT1")
        # yT, fT2 and outT share one rotating slot (disjoint live ranges)
        yT = fp.tile([P, DS, TQ], F32R, tag="fT2", name="yT")
        yTv = yT[:, 0:ES, :]

        # LN2 + transpose
        for qi in range(NQ):
            xt = xtp.tile([P, E], F32, tag="xt")
            nc.sync.dma_start(xt[:], x1s[qi * P : (qi + 1) * P, :])
            ln_transpose(xt, yTv, qi * P, t_g2, t_b2)
        # fin: fT1 = relu(finw.T @ yT + finb)
        for mi in range(DS):
            wc = wp.tile([P, ES, P], F32R, tag="w")
            nc.sync.dma_start(wc[:], finw[mi].rearrange("p (e j) -> p e j", e=ES))
            psm = ps.tile([P, 512], F32, tag="ps")
            for es in range(ES):
                nc.tensor.matmul(
                    psm[:],
                    wc[:, es, :],
                    yTv[:, es, :],
                    start=(es == 0),
                    stop=(es == ES - 1),
                )
            nc.scalar.activation(
                fT1[:, mi, :], psm[:], AF.Relu, bias=t_finb[:, mi : mi + 1]
            )

        if debug:
            for es in range(ES):
                nc.sync.dma_start(dbg["d_yt"][es * P:(es + 1) * P, :], yTv[:, es, :].bitcast(F32))
            for mi in range(DS):
                nc.sync.dma_start(dbg["d_ft1"][mi * P:(mi + 1) * P, :], fT1[:, mi, :].bitcast(F32))

        # two hidden layers: fT1 -> fT2 -> fT1b (tags rotate within the pool)
        hb = t_hidb[:].rearrange("p (l d) -> p l d", l=2)

        def dense_layer(fin_t, fout_t, w_dram, li, n_m, bias_ap, func):
            for mi in range(n_m):
                psm = ps.tile([P, 512], F32, tag="ps")
                for kq in range(4):
                    whc = wp.tile([P, ES, P], F32R, tag="w")
                    src = w_dram[li, mi, kq] if li is not None else w_dram[mi, kq]
                    nc.sync.dma_start(
                        whc[:], src.rearrange("p (k j) -> p k j", k=ES)
                    )
                    for ks in range(ES):
                        nc.tensor.matmul(
                            psm[:],
                            whc[:, ks, :],
                            fin_t[:, kq * ES + ks, :],
                            start=(kq == 0 and ks == 0),
                            stop=(kq == 3 and ks == ES - 1),
                        )
                nc.scalar.activation(
                    fout_t[:, mi, :], psm[:], func, bias=bias_ap[:, mi : mi + 1]
                )

        fT2 = fp.tile([P, DS, TQ], F32R, tag="fT2", name="fT2")
        dense_layer(fT1, fT2, hidw, 0, DS, hb[:, 0, :], AF.Relu)
        if debug:
            for mi in range(DS):
                nc.sync.dma_start(dbg["d_ft2"][mi * P:(mi + 1) * P, :], fT2[:, mi, :].bitcast(F32))
        fT1b = fp.tile([P, DS, TQ], F32R, tag="fT1", name="fT1b")
        dense_layer(fT2, fT1b, hidw, 1, DS, hb[:, 1, :], AF.Relu)
        if debug:
            for mi in range(DS):
                nc.sync.dma_start(dbg["d_ft3"][mi * P:(mi + 1) * P, :], fT1b[:, mi, :].bitcast(F32))
        outT = fp.tile([P, DS, TQ], F32R, tag="fT2", name="outT")
        outTv = outT[:, 0:ES, :]
        dense_layer(fT1b, outTv, foutw, None, ES, t_foutb, AF.Identity)

        if debug:
            for es in range(ES):
                nc.sync.dma_start(dbg["d_outt"][es * P:(es + 1) * P, :], outTv[:, es, :].bitcast(F32))
            nc.sync.dma_start(dbg["d_finb"][:], t_finb[:])
            nc.sync.dma_start(dbg["d_hidb"][:], t_hidb[:])
            nc.sync.dma_start(dbg["d_projb"][:], t_projb[:])
            nc.sync.dma_start(dbg["d_foutb"][:], t_foutb[:])
            nc.sync.dma_start(dbg["d_g2"][:], t_g2[:])
        # out = x1 + outT^T
        for qi in range(NQ):
            xr = xtp.tile([P, E], F32, tag="xt", name=f"xr{qi}")
            nc.sync.dma_start(xr[:], x1s[qi * P : (qi + 1) * P, :])
            orow = xtp.tile([P, E], F32, tag="x1row", name=f"orow{qi}")
            for es in range(ES):
                ptp = tp.tile([P, P], F32R, tag="tp")
                nc.tensor.transpose(
                    ptp[:], outTv[:, es, qi * P : (qi + 1) * P], t_ident[:]
                )
                nc.vector.tensor_tensor(
                    orow[:, es * P : (es + 1) * P],
                    ptp[:],
                    xr[:, es * P : (es + 1) * P],
                    OP.add,
                )
            nc.sync.dma_start(out[qi * P : (qi + 1) * P, :], orow[:])

        ffn_cm.__exit__(None, None, None)
        for cm in reversed(pools):
            cm.__exit__(None, None, None)

    return nc


# ---------------------------------------------------------------------------
# Host-side input prep
# ---------------------------------------------------------------------------
def _prep_shared(inputs):
    f = np.float32
    asf = lambda a: np.ascontiguousarray(np.asarray(a, f))

    Wq = asf(inputs["Wq"]).transpose(1, 0, 2).reshape(E, H * KD)
    Wk = asf(inputs["Wk"]).transpose(1, 0, 2).reshape(E, H * KD)
    Wv = asf(inputs["Wv"]).transpose(1, 0, 2).reshape(E, H * KD)
    projW = asf(inputs["proj_W"])
    finW = asf(inputs["fin_W"])
    hidW = asf(inputs["hid_W"])
    foutW = asf(inputs["fout_W"])

    def lhst_1024(Wm):  # [1024, 1024] -> [mi=8, p=128, es*jj=1024]
        return np.ascontiguousarray(
            Wm.reshape(ES, P, ES, P).transpose(2, 1, 0, 3).reshape(ES, P, ES * P)
        )

    shared = {
        "wq": lhst_1024(Wq),
        "wk": lhst_1024(Wk),
        "projw": lhst_1024(projW),
        # wv as rhs quarters: [jq=4, p, es*256]
        "wv": np.ascontiguousarray(
            Wv.reshape(ES, P, 4, 256).transpose(2, 1, 0, 3).reshape(4, P, ES * 256)
        ),
        # finW [1024, 4096] -> [mi=32, p, es*128]
        "finw": np.ascontiguousarray(
            finW.reshape(ES, P, DS, P).transpose(2, 1, 0, 3).reshape(DS, P, ES * P)
        ),
        # hidW [2, 4096, 4096] -> [li, mi=32, kq=4, p, 8*128]
        "hidw": np.ascontiguousarray(
            hidW.reshape(2, 4, ES, P, DS, P)
            .transpose(0, 4, 1, 3, 2, 5)
            .reshape(2, DS, 4, P, ES * P)
        ),
        # foutW [4096, 1024] -> [mi=8, kq=4, p, 8*128]
        "foutw": np.ascontiguousarray(
            foutW.reshape(4, ES, P, ES, P)
            .transpose(3, 0, 2, 1, 4)
            .reshape(ES, 4, P, ES * P)
        ),
        "ident": np.eye(P, dtype=f),
        "trilc": np.tril(np.ones((P, P), f)).T.copy(),  # A[j,tk] = [tk >= j]
        "onesc": np.ones((P, 64), f),
        "g1c": asf(inputs["ln1_g"]).reshape(ES, P).T.copy(),
        "b1c": asf(inputs["ln1_b"]).reshape(ES, P).T.copy(),
        "g2c": asf(inputs["ln2_g"]).reshape(ES, P).T.copy(),
        "b2c": asf(inputs["ln2_b"]).reshape(ES, P).T.copy(),
        "projb": asf(inputs["proj_b"]).reshape(ES, P).T.copy(),
        "finb": asf(inputs["fin_b"]).reshape(DS, P).T.copy(),
        "hidb": np.ascontiguousarray(
            asf(inputs["hid_b"]).reshape(2, DS, P).transpose(2, 0, 1).reshape(P, 2 * DS)
        ),
        "foutb": asf(inputs["fout_b"]).reshape(ES, P).T.copy(),
    }
    return shared


def _mask_for(qstart):
    # One-hot "B" operand for the tril-matmul mask: with A[j,tk] = [tk >= j],
    # (A.T @ B)[tk, c] = NEG iff tk >= s(c), s(c) = qstart + c - ko*P + 1.
    m = np.zeros((P, KO, TQ), np.float32)
    for ko in range(KO):
        for c in range(TQ):
            s = qstart + c - ko * P + 1
            if s <= 0:
                m[0, ko, c] = NEG
            elif s <= P - 1:
                m[s, ko, c] = NEG
    return np.ascontiguousarray(m.reshape(P, KO * TQ))


_cached = {}


def kernel(**inputs):
    if "nc" not in _cached:
        _cached["nc"] = build_program()
    nc = _cached["nc"]

    from concourse import bass_utils

    x = np.ascontiguousarray(np.asarray(inputs["x"], np.float32))
    shared = _prep_shared(inputs)
    masks = [_mask_for(0), _mask_for(TQ)]

    in_maps = []
    for c in range(8):
        b, h = c // 2, c % 2
        m = dict(shared)
        m["x_kv"] = np.ascontiguousarray(x[b])
        m["x_q"] = np.ascontiguousarray(x[b, h * TQ : (h + 1) * TQ, :])
        m["maskt"] = masks[h]
        in_maps.append(m)

    res = bass_utils.run_bass_kernel_spmd(nc, in_maps, core_ids=list(range(8)))
    outs = res.results
    y = np.empty((B, S, E), np.float32)
    for c in range(8):
        b, h = c // 2, c % 2
        y[b, h * TQ : (h + 1) * TQ, :] = outs[c]["out"]
    return y

